# revision 22
# baseline (speedup 1.0000x reference)
"""Trainium2 Bass kernel for nn_Attention_Mod (B=4, C=512, H=W=64, Cq=64).

out = gamma * (V @ softmax(Q K^T over keys)^T) + x

Sharding: 8 cores = 4 batches x 2 query-halves. Each core computes attention
for 2048 queries of one batch against all 4096 keys. Per-core inputs are the
batch's x (columns rotated so the core's query half comes first) in f16, its
transpose in bf16, plus replicated weights (gamma folded into Wv^T).

v3 design notes (PE-cycle minimization; cost per matmul = free-dim columns):
 - softmax over keys without a row-max pass: energies are bounded (|E|<~110)
   so exp(E - 64) stays in fp32/bf16 range and the ratio is unchanged.
 - the whole projection/energy path runs in float16 (x, W packs, k, q):
   f16 streams at 1 col/cycle like bf16 (fp32r measured ~2x slower per
   column on HW), and f16's 10 mantissa bits make the energy path MORE
   accurate than bf16 (measured rel_l2 3e-3 vs 9e-3).
 - energy matmuls have contraction Cq=64 only: they run as ROW-TILED pairs
   (tile_position (0,0)/(64,0)), two concurrent 64x128 tiles in the PE
   array. Pairs are emitted in groups of two (4 chunks) sharing a merged
   4-bank psum ring so the 64<->128 tile-mode transition cost (~200ns) is
   amortized over 4 chunks instead of 2. k and q are duplicated on both
   partition halves via the [W|W] duplicated-column projection packs.
 - PV is reassociated: yu = x . exp^T accumulates over key chunks (lhsT =
   x^T chunks in bf16), then out = (gamma Wv^T)^T . yu applies the value
   projection to only this core's 2048 query columns instead of all 4096
   keys (halves the value-projection matmul cycles vs projecting V).
 - the softmax normalizer (column sum over keys) accumulates on the vector
   engine in fp32; an all-ones [128,128] bf16 stationary matmul broadcasts
   it across partitions in one shot; out = psum * recip + x.
 - block 0's energies AND yu accumulation run inside the x streaming loop
   so the PE is saturated while the input DMA streams.
 - tail: block 3's yu copies run on the (idle) scalar engine and its output
   DMAs are split in half across queues to shorten the exposed drain.
"""

import numpy as np
from contextlib import ExitStack

B, C, H, W = 4, 512, 64, 64
N = H * W           # 4096 keys
NH = N // 2         # 2048 queries per core
CQ = 64
P = 128
CC = C // P         # 4 contraction chunks
MB = N // P         # 32 key chunks
NBLK = NH // 512    # 4 query blocks of 512
DB = C // P         # 4 output-channel blocks
NCORES = 8
SHIFT = 64.0
WARMUP_MM = 8       # dummy matmuls to lift the PE HAM clock gate at start
PVLAG = 4           # PV emission lags energy by this many key chunks

_compiled = None
_RUN_KWARGS = {}   # test harness may set dict(trace=True, ...)
_LAST = None       # last BassKernelResults, for the test harness


def _build():
    import concourse.bass as bass
    from concourse import bacc
    import concourse.tile as tile
    from concourse import mybir

    f32 = mybir.dt.float32
    f16 = mybir.dt.float16
    bf16 = mybir.dt.bfloat16
    ts = bass.ts

    nc = bacc.Bacc("TRN2", target_bir_lowering=False, debug=False)
    # per-core inputs; wq2/wk2 are [W|W] duplicated-column packs; xt is the
    # full rotated x pre-transposed ([keys, C], bf16) for the yu matmuls
    xb_d = nc.dram_tensor("xb", [C, N], f16, kind="ExternalInput").ap()
    xt_d = nc.dram_tensor("xt", [N, C], bf16, kind="ExternalInput").ap()
    wq2_d = nc.dram_tensor("wq2", [C, P], f16, kind="ExternalInput").ap()
    wk2_d = nc.dram_tensor("wk2", [C, P], f16, kind="ExternalInput").ap()
    wv_d = nc.dram_tensor("wvT", [C, C], bf16, kind="ExternalInput").ap()
    ones_d = nc.dram_tensor("ones", [P, P], bf16, kind="ExternalInput").ap()
    out_d = nc.dram_tensor("out", [C, NH], bf16, kind="ExternalOutput").ap()

    with tile.TileContext(nc) as tc, ExitStack() as ctx:
        big = ctx.enter_context(tc.tile_pool(name="big", bufs=1))
        expp = ctx.enter_context(tc.tile_pool(name="expp", bufs=12))
        yusb = ctx.enter_context(tc.tile_pool(name="yusb", bufs=8))
        outst = ctx.enter_context(tc.tile_pool(name="outst", bufs=4))
        scal = ctx.enter_context(tc.tile_pool(name="scal", bufs=1))
        yup = ctx.enter_context(tc.tile_pool(name="yup", bufs=4, space="PSUM"))
        mp = ctx.enter_context(tc.tile_pool(name="mp", bufs=4, space="PSUM"))

        xf = big.tile([P, CC, N], f16)
        xb_r = xb_d.rearrange("(cc p) n -> p cc n", p=P)
        xt_r = xt_d.rearrange("(m p) c -> p m c", p=P)
        # x block 0 first: it gates the first projection matmuls
        nc.sync.dma_start(xf[:, :, ts(0, 512)], xb_r[:, :, ts(0, 512)])

        # ---- PE warm-up: open the HAM clock gate while DMAs stream ----
        wsrc = big.tile([P, 512], bf16)
        nc.vector.memset(wsrc[:], 1.0)
        wps = mp.tile([P, 512], f32, tag="mp", name="warm_ps")
        for _ in range(WARMUP_MM):
            nc.tensor.matmul(wps[:], lhsT=wsrc[:, 0:P], rhs=wsrc[:],
                             start=True, stop=True)

        # ---- small loads up front ----
        wk2_sb = big.tile([P, CC, P], f16)
        nc.sync.dma_start(wk2_sb[:], wk2_d.rearrange("(cc p) q -> p cc q", p=P))
        wq2_sb = big.tile([P, CC, P], f16)
        nc.sync.dma_start(wq2_sb[:], wq2_d.rearrange("(cc p) q -> p cc q", p=P))
        ones_sb = big.tile([P, P], bf16)
        shift_sb = big.tile([P, 1], f32)
        nc.vector.memset(shift_sb[:], -SHIFT)
        wv_sb = big.tile([P, CC, C], bf16)

        ks2 = big.tile([P, N], f16)       # [k; k]: duplicated halves
        qd = big.tile([P, NH], f16)       # [q; q]: duplicated halves
        xtt = big.tile([P, MB, C], bf16)  # x^T per 128-key chunk, bf16

        out_r = out_d.rearrange("(db p) n -> p db n", p=P)

        # ---- bookkeeping for the pipelined attention stream ----
        csum_t = {}     # nb -> csum tile
        yus = {}        # nb -> [4 psum accumulators]
        ex_map = {}     # (nb, mc) -> exp tile
        pv_queue = []   # (nb, mc) chunks whose PV is not yet emitted
        cs_pend = []    # (nb, mc, ex) whose csum add is not yet emitted
        sbc_t = {}      # nb -> reciprocal normalizer tile

        def emit_energy(nb, mc, lo):
            # one row-tiled energy matmul: contraction rows [lo, lo+64)
            e = mp_tile(f"e{nb}_{mc}")
            nc.tensor.matmul(e[:], lhsT=ks2[lo:lo + CQ, ts(mc, P)],
                             rhs=qd[lo:lo + CQ, ts(nb, 512)],
                             start=True, stop=True)
            ex = expp.tile([P, 512], bf16, tag="ex", name=f"ex{nb}_{mc}",
                           bufs=12)
            nc.scalar.activation(
                out=ex[:], in_=e[:],
                func=mybir.ActivationFunctionType.Exp,
                bias=shift_sb[:], scale=1.0)
            ex_map[(nb, mc)] = ex
            pv_queue.append((nb, mc))
            cs_pend.append((nb, mc, ex))

        def flush_csum(keep):
            # csum adds are deferred a few chunks so the DVE queue never
            # blocks behind the exp chain (that stalls psum-bank recycling
            # and splits the concurrent energy pairs)
            while len(cs_pend) > keep:
                nb, mc, ex = cs_pend.pop(0)
                if mc == 0:
                    csum_t[nb] = scal.tile([P, 512], bf16, tag="csum",
                                           name=f"csum{nb}", bufs=2)
                    nc.vector.tensor_copy(csum_t[nb][:], ex[:])
                else:
                    nc.vector.tensor_add(csum_t[nb][:], csum_t[nb][:], ex[:])

        mp_allocs = [1]   # the warmup psum tile already took a slot

        def mp_tile(name):
            mp_allocs[0] += 1
            return mp.tile([P, 512], f32, tag="mp", name=name)

        def mp_pad():
            # dummy ring allocations keep the pair tiles phase-aligned with
            # slots whose exp reads are long done (else pairs split)
            while mp_allocs[0] % 4:
                mp_tile(f"pad{mp_allocs[0]}")

        def emit_pair(nb, t):
            emit_energy(nb, 2 * t, 0)
            emit_energy(nb, 2 * t + 1, CQ)

        def emit_pv(n):
            for _ in range(n):
                if not pv_queue:
                    return
                nb, mc = pv_queue.pop(0)
                ex = ex_map.pop((nb, mc))
                if mc == 0:
                    yus[nb] = [yup.tile([P, 512], f32, tag="yu",
                                        name=f"yu{nb}_{d}") for d in range(DB)]
                for db in range(DB):
                    nc.tensor.matmul(
                        yus[nb][db][:], lhsT=xtt[:, mc, ts(db, P)],
                        rhs=ex[:], start=(mc == 0), stop=(mc == MB - 1))

        def emit_csnorm(nb):
            # broadcast the key-sum across partitions with an all-ones
            # stationary matmul, then take the reciprocal
            assert not any(p[0] == nb for p in cs_pend)
            csr = scal.tile([P, 512], bf16, tag="csr", name=f"csr{nb}",
                            bufs=2)
            nc.vector.tensor_copy(csr[:], csum_t[nb][:])
            cs_ps = mp_tile(f"cs{nb}")
            nc.tensor.matmul(cs_ps[:], lhsT=ones_sb[:], rhs=csr[:],
                             start=True, stop=True)
            sbc = scal.tile([P, 512], f32, tag="sbc", name=f"sbc{nb}",
                            bufs=2)
            nc.vector.reciprocal_approx_fast(sbc[:], cs_ps[:])
            sbc_t[nb] = sbc

        def emit_yu_norm(nb):
            # normalization folded into the psum->sbuf copy: yc = yu * recip,
            # so only the residual add remains after the Wv matmuls
            cps = []
            for db in range(DB):
                yc = yusb.tile([P, 512], bf16, tag="yc", name=f"yc{nb}_{db}",
                               bufs=8)
                nc.vector.tensor_mul(yc[:], yus[nb][db][:], sbc_t[nb][:])
                cps.append(yc)
            return cps

        def emit_wv(nb, ycs, tail=False):
            # out[d] = sum_cc WvT[cc, d-block]^T . yu[cc]; normalize + residual
            for db in range(DB):
                ops = mp_tile(f"o{nb}_{db}")
                for cc in range(CC):
                    nc.tensor.matmul(ops[:], lhsT=wv_sb[:, cc, ts(db, P)],
                                     rhs=ycs[cc][:],
                                     start=(cc == 0), stop=(cc == CC - 1))
                t = outst.tile([P, 512], bf16, tag="t", name=f"t{nb}_{db}",
                               bufs=4)
                nc.vector.tensor_add(t[:], ops[:], xf[:, db, ts(nb, 512)])
                if not tail:
                    nc.sync.dma_start(out_r[:, db, ts(nb, 512)], t[:])
                elif db < DB - 1:
                    # alternate issue engines so the drain isn't serialized
                    # on one queue's ~0.6us per-descriptor issue cost
                    eng = nc.scalar if db % 2 else nc.sync
                    eng.dma_start(out_r[:, db, ts(nb, 512)], t[:])
                else:
                    nc.sync.dma_start(
                        out_r[:, db, 512 * nb:512 * nb + 256], t[:, 0:256])
                    nc.scalar.dma_start(
                        out_r[:, db, 512 * nb + 256:512 * nb + 512],
                        t[:, 256:512])

        # ---- streamed projections + block-0 attention ----
        for mb in range(N // 512):
            if mb + 1 < N // 512:
                # prefetch the NEXT x block ahead of this iteration's x^T:
                # xf gates the projections, x^T only next iteration's PV
                nc.sync.dma_start(xf[:, :, ts(mb + 1, 512)],
                                  xb_r[:, :, ts(mb + 1, 512)])
            nc.sync.dma_start(xtt[:, 4 * mb:4 * mb + 4, :],
                              xt_r[:, 4 * mb:4 * mb + 4, :])
            if mb == 2:
                nc.sync.dma_start(wv_sb[:],
                                  wv_d.rearrange("(cc p) d -> p cc d", p=P))
                nc.sync.dma_start(ones_sb[:], ones_d)

            # block-0 energies + PV one x-block behind the stream, pairs
            # grouped by two to amortize the 64<->128 tile-mode switch.
            # Pairs go FIRST so their psum slots reuse the PREVIOUS
            # iteration's proj tiles, whose casts are long done (else the
            # pair splits waiting on the cast and pays extra mode switches).
            if mb >= 1:
                emit_pair(0, 2 * (mb - 1))
                emit_pair(0, 2 * (mb - 1) + 1)
                emit_pv(len(pv_queue) - PVLAG)
                flush_csum(4)

            # q/k blocks: single f16 pass each; psum rows are duplicated
            while mp_allocs[0] % 4 != 4 - (2 if mb < NBLK else 1):
                mp_tile(f"pad{mb}_{mp_allocs[0]}")
            if mb < NBLK:
                psq = mp_tile(f"qp{mb}")
                for cc in range(CC):
                    nc.tensor.matmul(
                        psq[:], lhsT=wq2_sb[:, cc, :],
                        rhs=xf[:, cc, ts(mb, 512)],
                        start=(cc == 0), stop=(cc == CC - 1))
                nc.vector.tensor_copy(qd[:, ts(mb, 512)], psq[:])

            ps = mp_tile(f"kp{mb}")
            for cc in range(CC):
                nc.tensor.matmul(
                    ps[:], lhsT=wk2_sb[:, cc, :], rhs=xf[:, cc, ts(mb, 512)],
                    start=(cc == 0), stop=(cc == CC - 1))
            nc.vector.tensor_copy(ks2[:, ts(mb, 512)], ps[:])

        # ---- attention: remaining pairs of block 0, then blocks 1-3 ----
        ycs_pend = {}
        for nb in range(NBLK):
            for t in range(14 if nb == 0 else 0, 16, 2):
                emit_pair(nb, t)
                emit_pair(nb, t + 1)
                emit_pv(len(pv_queue) - PVLAG)
                flush_csum(4)
                if nb >= 1:
                    prev = nb - 1
                    if t == 0:
                        # previous block's PV just drained: free its psum
                        emit_csnorm(prev)
                        mp_pad()
                        ycs_pend[prev] = emit_yu_norm(prev)
                    if t == 2:
                        emit_wv(prev, ycs_pend.pop(prev))

        # ---- tail: drain block 3 ----
        emit_pv(2)
        flush_csum(0)
        emit_csnorm(NBLK - 1)
        emit_pv(len(pv_queue))
        ycs = emit_yu_norm(NBLK - 1)
        emit_wv(NBLK - 1, ycs, tail=True)

    nc.compile()
    return nc


def _get_compiled():
    global _compiled
    if _compiled is None:
        _compiled = _build()
    return _compiled


def kernel(x, Wq, Wk, Wv, gamma, **_unused):
    import ml_dtypes
    from concourse import bass_utils

    x = np.asarray(x, dtype=np.float32)
    Wq = np.asarray(Wq, dtype=np.float32)
    Wk = np.asarray(Wk, dtype=np.float32)
    Wv = np.asarray(Wv, dtype=np.float32)
    gamma = np.asarray(gamma, dtype=np.float32)

    xf = x.reshape(B, C, N)

    # [W|W] duplicated-column packs: the projection PSUM holds the value
    # duplicated on partitions 0:64 / 64:128
    def pack2(Wm):
        wT = np.ascontiguousarray(Wm.T)          # [C, CQ]
        return np.ascontiguousarray(
            np.concatenate([wT, wT], axis=1)).astype(np.float16)

    wq2 = pack2(Wq)
    wk2 = pack2(Wk)
    wvT = (np.ascontiguousarray(Wv.T) * gamma[0]).astype(ml_dtypes.bfloat16)
    ones = np.ones((P, P), dtype=ml_dtypes.bfloat16)

    in_maps = []
    for core in range(NCORES):
        b, half = core // 2, core % 2
        xb = xf[b]
        if half:
            xb = np.concatenate([xb[:, NH:], xb[:, :NH]], axis=1)
        xb = np.ascontiguousarray(xb)
        xt = np.ascontiguousarray(xb.T).astype(ml_dtypes.bfloat16)
        in_maps.append({"xb": xb.astype(np.float16), "xt": xt, "wq2": wq2,
                        "wk2": wk2, "wvT": wvT, "ones": ones})

    nc = _get_compiled()
    res = bass_utils.run_bass_kernel_spmd(
        nc, in_maps, core_ids=list(range(NCORES)), **_RUN_KWARGS
    )
    global _LAST
    _LAST = res

    out = np.empty((B, C, N), dtype=np.float32)
    for core in range(NCORES):
        b, half = core // 2, core % 2
        lo = half * NH
        out[b][:, lo:lo + NH] = res.results[core]["out"].astype(np.float32)
    return out.reshape(B, C, H, W)


# revision 23
# speedup vs baseline: 1.0071x; 1.0071x over previous
"""Trainium2 Bass kernel for nn_Attention_Mod (B=4, C=512, H=W=64, Cq=64).

out = gamma * (V @ softmax(Q K^T over keys)^T) + x

Sharding: 8 cores = 4 batches x 2 query-halves. Each core computes attention
for 2048 queries of one batch against all 4096 keys. Per-core inputs are the
batch's x (columns rotated so the core's query half comes first) in f16, its
transpose in bf16, plus replicated weights (gamma folded into Wv^T).

v3 design notes (PE-cycle minimization; cost per matmul = free-dim columns):
 - softmax over keys without a row-max pass: energies are bounded (|E|<~110)
   so exp(E - 64) stays in fp32/bf16 range and the ratio is unchanged.
 - the whole projection/energy path runs in float16 (x, W packs, k, q):
   f16 streams at 1 col/cycle like bf16 (fp32r measured ~2x slower per
   column on HW), and f16's 10 mantissa bits make the energy path MORE
   accurate than bf16 (measured rel_l2 3e-3 vs 9e-3).
 - energy matmuls have contraction Cq=64 only: they run as ROW-TILED pairs
   (tile_position (0,0)/(64,0)), two concurrent 64x128 tiles in the PE
   array. Pairs are emitted in groups of two (4 chunks) sharing a merged
   4-bank psum ring so the 64<->128 tile-mode transition cost (~200ns) is
   amortized over 4 chunks instead of 2. k and q are duplicated on both
   partition halves via the [W|W] duplicated-column projection packs.
 - PV is reassociated: yu = x . exp^T accumulates over key chunks (lhsT =
   x^T chunks in bf16), then out = (gamma Wv^T)^T . yu applies the value
   projection to only this core's 2048 query columns instead of all 4096
   keys (halves the value-projection matmul cycles vs projecting V).
 - the softmax normalizer (column sum over keys) accumulates on the vector
   engine in fp32; an all-ones [128,128] bf16 stationary matmul broadcasts
   it across partitions in one shot; out = psum * recip + x.
 - block 0's energies AND yu accumulation run inside the x streaming loop
   so the PE is saturated while the input DMA streams.
 - tail: block 3's yu copies run on the (idle) scalar engine and its output
   DMAs are split in half across queues to shorten the exposed drain.
"""

import numpy as np
from contextlib import ExitStack

B, C, H, W = 4, 512, 64, 64
N = H * W           # 4096 keys
NH = N // 2         # 2048 queries per core
CQ = 64
P = 128
CC = C // P         # 4 contraction chunks
MB = N // P         # 32 key chunks
NBLK = NH // 512    # 4 query blocks of 512
DB = C // P         # 4 output-channel blocks
NCORES = 8
SHIFT = 64.0
WARMUP_MM = 8       # dummy matmuls to lift the PE HAM clock gate at start
PVLAG = 4           # PV emission lags energy by this many key chunks

_compiled = None
_RUN_KWARGS = {}   # test harness may set dict(trace=True, ...)
_LAST = None       # last BassKernelResults, for the test harness


def _build():
    import concourse.bass as bass
    from concourse import bacc
    import concourse.tile as tile
    from concourse import mybir

    f32 = mybir.dt.float32
    f16 = mybir.dt.float16
    bf16 = mybir.dt.bfloat16
    ts = bass.ts

    nc = bacc.Bacc("TRN2", target_bir_lowering=False, debug=False)
    # per-core inputs; wq2/wk2 are [W|W] duplicated-column packs; xt is the
    # full rotated x pre-transposed ([keys, C], bf16) for the yu matmuls
    xb_d = nc.dram_tensor("xb", [C, N], f16, kind="ExternalInput").ap()
    xt_d = nc.dram_tensor("xt", [N, C], bf16, kind="ExternalInput").ap()
    wq2_d = nc.dram_tensor("wq2", [C, P], f16, kind="ExternalInput").ap()
    wk2_d = nc.dram_tensor("wk2", [C, P], f16, kind="ExternalInput").ap()
    wv_d = nc.dram_tensor("wvT", [C, C], bf16, kind="ExternalInput").ap()
    ones_d = nc.dram_tensor("ones", [P, P], bf16, kind="ExternalInput").ap()
    out_d = nc.dram_tensor("out", [C, NH], bf16, kind="ExternalOutput").ap()

    with tile.TileContext(nc) as tc, ExitStack() as ctx:
        big = ctx.enter_context(tc.tile_pool(name="big", bufs=1))
        expp = ctx.enter_context(tc.tile_pool(name="expp", bufs=12))
        yusb = ctx.enter_context(tc.tile_pool(name="yusb", bufs=8))
        outst = ctx.enter_context(tc.tile_pool(name="outst", bufs=4))
        scal = ctx.enter_context(tc.tile_pool(name="scal", bufs=1))
        yup = ctx.enter_context(tc.tile_pool(name="yup", bufs=4, space="PSUM"))
        mp = ctx.enter_context(tc.tile_pool(name="mp", bufs=4, space="PSUM"))

        xf = big.tile([P, CC, N], f16)
        xb_r = xb_d.rearrange("(cc p) n -> p cc n", p=P)
        xt_r = xt_d.rearrange("(m p) c -> p m c", p=P)
        # x block 0 first: it gates the first projection matmuls
        nc.sync.dma_start(xf[:, :, ts(0, 512)], xb_r[:, :, ts(0, 512)])

        # ---- PE warm-up: open the HAM clock gate while DMAs stream ----
        wsrc = big.tile([P, 512], bf16)
        nc.vector.memset(wsrc[:], 1.0)
        wps = mp.tile([P, 512], f32, tag="mp", name="warm_ps")
        for _ in range(WARMUP_MM):
            nc.tensor.matmul(wps[:], lhsT=wsrc[:, 0:P], rhs=wsrc[:],
                             start=True, stop=True)

        # ---- small loads up front ----
        wk2_sb = big.tile([P, CC, P], f16)
        nc.sync.dma_start(wk2_sb[:], wk2_d.rearrange("(cc p) q -> p cc q", p=P))
        wq2_sb = big.tile([P, CC, P], f16)
        nc.sync.dma_start(wq2_sb[:], wq2_d.rearrange("(cc p) q -> p cc q", p=P))
        ones_sb = big.tile([P, P], bf16)
        shift_sb = big.tile([P, 1], f32)
        nc.vector.memset(shift_sb[:], -SHIFT)
        wv_sb = big.tile([P, CC, C], bf16)

        ks2 = big.tile([P, N], f16)       # [k; k]: duplicated halves
        qd = big.tile([P, NH], f16)       # [q; q]: duplicated halves
        xtt = big.tile([P, MB, C], bf16)  # x^T per 128-key chunk, bf16

        out_r = out_d.rearrange("(db p) n -> p db n", p=P)

        # ---- bookkeeping for the pipelined attention stream ----
        csum_t = {}     # nb -> csum tile
        yus = {}        # nb -> [4 psum accumulators]
        ex_map = {}     # (nb, mc) -> exp tile
        pv_queue = []   # (nb, mc) chunks whose PV is not yet emitted
        cs_pend = []    # (nb, mc, ex) whose csum add is not yet emitted
        sbc_t = {}      # nb -> reciprocal normalizer tile

        def emit_energy(nb, mc, lo):
            # one row-tiled energy matmul: contraction rows [lo, lo+64)
            e = mp_tile(f"e{nb}_{mc}")
            nc.tensor.matmul(e[:], lhsT=ks2[lo:lo + CQ, ts(mc, P)],
                             rhs=qd[lo:lo + CQ, ts(nb, 512)],
                             start=True, stop=True)
            ex = expp.tile([P, 512], bf16, tag="ex", name=f"ex{nb}_{mc}",
                           bufs=12)
            nc.scalar.activation(
                out=ex[:], in_=e[:],
                func=mybir.ActivationFunctionType.Exp,
                bias=shift_sb[:], scale=1.0)
            ex_map[(nb, mc)] = ex
            pv_queue.append((nb, mc))
            cs_pend.append((nb, mc, ex))

        def flush_csum(keep):
            # csum adds are deferred a few chunks so the DVE queue never
            # blocks behind the exp chain (that stalls psum-bank recycling
            # and splits the concurrent energy pairs)
            while len(cs_pend) > keep:
                nb, mc, ex = cs_pend.pop(0)
                if mc == 0:
                    csum_t[nb] = scal.tile([P, 512], bf16, tag="csum",
                                           name=f"csum{nb}", bufs=2)
                    nc.vector.tensor_copy(csum_t[nb][:], ex[:])
                else:
                    nc.vector.tensor_add(csum_t[nb][:], csum_t[nb][:], ex[:])

        mp_allocs = [1]   # the warmup psum tile already took a slot

        def mp_tile(name):
            mp_allocs[0] += 1
            return mp.tile([P, 512], f32, tag="mp", name=name)

        def mp_pad():
            # dummy ring allocations keep the pair tiles phase-aligned with
            # slots whose exp reads are long done (else pairs split)
            while mp_allocs[0] % 4:
                mp_tile(f"pad{mp_allocs[0]}")

        def emit_pair(nb, t):
            emit_energy(nb, 2 * t, 0)
            emit_energy(nb, 2 * t + 1, CQ)

        def emit_pv(n):
            for _ in range(n):
                if not pv_queue:
                    return
                nb, mc = pv_queue.pop(0)
                ex = ex_map.pop((nb, mc))
                if mc == 0:
                    yus[nb] = [yup.tile([P, 512], f32, tag="yu",
                                        name=f"yu{nb}_{d}") for d in range(DB)]
                for db in range(DB):
                    nc.tensor.matmul(
                        yus[nb][db][:], lhsT=xtt[:, mc, ts(db, P)],
                        rhs=ex[:], start=(mc == 0), stop=(mc == MB - 1))

        def emit_csnorm(nb):
            # broadcast the key-sum across partitions with an all-ones
            # stationary matmul, then take the reciprocal
            assert not any(p[0] == nb for p in cs_pend)
            csr = scal.tile([P, 512], bf16, tag="csr", name=f"csr{nb}",
                            bufs=2)
            nc.vector.tensor_copy(csr[:], csum_t[nb][:])
            cs_ps = mp_tile(f"cs{nb}")
            nc.tensor.matmul(cs_ps[:], lhsT=ones_sb[:], rhs=csr[:],
                             start=True, stop=True)
            sbc = scal.tile([P, 512], f32, tag="sbc", name=f"sbc{nb}",
                            bufs=2)
            nc.vector.reciprocal_approx_fast(sbc[:], cs_ps[:])
            sbc_t[nb] = sbc

        def emit_yu_norm(nb):
            # normalization folded into the psum->sbuf copy: yc = yu * recip,
            # so only the residual add remains after the Wv matmuls
            cps = []
            for db in range(DB):
                yc = yusb.tile([P, 512], bf16, tag="yc", name=f"yc{nb}_{db}",
                               bufs=8)
                nc.vector.tensor_mul(yc[:], yus[nb][db][:], sbc_t[nb][:])
                cps.append(yc)
            return cps

        def emit_wv(nb, ycs, tail=False):
            # out[d] = sum_cc WvT[cc, d-block]^T . yu[cc]; normalize + residual
            for db in range(DB):
                ops = mp_tile(f"o{nb}_{db}")
                for cc in range(CC):
                    nc.tensor.matmul(ops[:], lhsT=wv_sb[:, cc, ts(db, P)],
                                     rhs=ycs[cc][:],
                                     start=(cc == 0), stop=(cc == CC - 1))
                t = outst.tile([P, 512], bf16, tag="t", name=f"t{nb}_{db}",
                               bufs=4)
                nc.vector.tensor_add(t[:], ops[:], xf[:, db, ts(nb, 512)])
                if not tail:
                    nc.sync.dma_start(out_r[:, db, ts(nb, 512)], t[:])
                elif db < DB - 1:
                    # alternate issue engines so the drain isn't serialized
                    # on one queue's ~0.6us per-descriptor issue cost
                    eng = nc.scalar if db % 2 else nc.sync
                    eng.dma_start(out_r[:, db, ts(nb, 512)], t[:])
                else:
                    nc.sync.dma_start(
                        out_r[:, db, 512 * nb:512 * nb + 256], t[:, 0:256])
                    nc.scalar.dma_start(
                        out_r[:, db, 512 * nb + 256:512 * nb + 512],
                        t[:, 256:512])

        # ---- streamed projections + block-0 attention ----
        for mb in range(N // 512):
            if mb + 1 < N // 512:
                # prefetch the NEXT x block ahead of this iteration's x^T:
                # xf gates the projections, x^T only next iteration's PV
                nc.sync.dma_start(xf[:, :, ts(mb + 1, 512)],
                                  xb_r[:, :, ts(mb + 1, 512)])
            nc.sync.dma_start(xtt[:, 4 * mb:4 * mb + 4, :],
                              xt_r[:, 4 * mb:4 * mb + 4, :])
            if mb == 2:
                nc.sync.dma_start(wv_sb[:],
                                  wv_d.rearrange("(cc p) d -> p cc d", p=P))
                nc.sync.dma_start(ones_sb[:], ones_d)

            # block-0 energies + PV one x-block behind the stream, pairs
            # grouped by two to amortize the 64<->128 tile-mode switch.
            # Pairs go FIRST so their psum slots reuse the PREVIOUS
            # iteration's proj tiles, whose casts are long done (else the
            # pair splits waiting on the cast and pays extra mode switches).
            if mb >= 1:
                emit_pair(0, 2 * (mb - 1))
                emit_pair(0, 2 * (mb - 1) + 1)
                emit_pv(len(pv_queue) - PVLAG)
                flush_csum(4)

            # k block: single f16 pass; psum rows are [k; k]
            while mp_allocs[0] % 4 != 4 - (2 if mb < NBLK else 1):
                mp_tile(f"pad{mb}_{mp_allocs[0]}")
            ps = mp_tile(f"kp{mb}")
            for cc in range(CC):
                nc.tensor.matmul(
                    ps[:], lhsT=wk2_sb[:, cc, :], rhs=xf[:, cc, ts(mb, 512)],
                    start=(cc == 0), stop=(cc == CC - 1))
            nc.vector.tensor_copy(ks2[:, ts(mb, 512)], ps[:])

            if mb < NBLK:
                psq = mp_tile(f"qp{mb}")
                for cc in range(CC):
                    nc.tensor.matmul(
                        psq[:], lhsT=wq2_sb[:, cc, :],
                        rhs=xf[:, cc, ts(mb, 512)],
                        start=(cc == 0), stop=(cc == CC - 1))
                nc.vector.tensor_copy(qd[:, ts(mb, 512)], psq[:])

        # ---- attention: remaining pairs of block 0, then blocks 1-3 ----
        ycs_pend = {}
        for nb in range(NBLK):
            for t in range(14 if nb == 0 else 0, 16, 2):
                emit_pair(nb, t)
                emit_pair(nb, t + 1)
                emit_pv(len(pv_queue) - PVLAG)
                flush_csum(4)
                if nb >= 1:
                    prev = nb - 1
                    if t == 0:
                        # previous block's PV just drained: free its psum
                        emit_csnorm(prev)
                        mp_pad()
                        ycs_pend[prev] = emit_yu_norm(prev)
                    if t == 2:
                        emit_wv(prev, ycs_pend.pop(prev))

        # ---- tail: drain block 3 ----
        emit_pv(2)
        flush_csum(0)
        emit_csnorm(NBLK - 1)
        emit_pv(len(pv_queue))
        ycs = emit_yu_norm(NBLK - 1)
        emit_wv(NBLK - 1, ycs, tail=True)

    nc.compile()
    return nc


def _get_compiled():
    global _compiled
    if _compiled is None:
        _compiled = _build()
    return _compiled


def kernel(x, Wq, Wk, Wv, gamma, **_unused):
    import ml_dtypes
    from concourse import bass_utils

    x = np.asarray(x, dtype=np.float32)
    Wq = np.asarray(Wq, dtype=np.float32)
    Wk = np.asarray(Wk, dtype=np.float32)
    Wv = np.asarray(Wv, dtype=np.float32)
    gamma = np.asarray(gamma, dtype=np.float32)

    xf = x.reshape(B, C, N)

    # [W|W] duplicated-column packs: the projection PSUM holds the value
    # duplicated on partitions 0:64 / 64:128
    def pack2(Wm):
        wT = np.ascontiguousarray(Wm.T)          # [C, CQ]
        return np.ascontiguousarray(
            np.concatenate([wT, wT], axis=1)).astype(np.float16)

    wq2 = pack2(Wq)
    wk2 = pack2(Wk)
    wvT = (np.ascontiguousarray(Wv.T) * gamma[0]).astype(ml_dtypes.bfloat16)
    ones = np.ones((P, P), dtype=ml_dtypes.bfloat16)

    in_maps = []
    for core in range(NCORES):
        b, half = core // 2, core % 2
        xb = xf[b]
        if half:
            xb = np.concatenate([xb[:, NH:], xb[:, :NH]], axis=1)
        xb = np.ascontiguousarray(xb)
        xt = np.ascontiguousarray(xb.T).astype(ml_dtypes.bfloat16)
        in_maps.append({"xb": xb.astype(np.float16), "xt": xt, "wq2": wq2,
                        "wk2": wk2, "wvT": wvT, "ones": ones})

    nc = _get_compiled()
    res = bass_utils.run_bass_kernel_spmd(
        nc, in_maps, core_ids=list(range(NCORES)), **_RUN_KWARGS
    )
    global _LAST
    _LAST = res

    out = np.empty((B, C, N), dtype=np.float32)
    for core in range(NCORES):
        b, half = core // 2, core % 2
        lo = half * NH
        out[b][:, lo:lo + NH] = res.results[core]["out"].astype(np.float32)
    return out.reshape(B, C, H, W)
